# revision 1
# baseline (speedup 1.0000x reference)
"""KNN anomaly-scoring kernel for Trainium2 (Bass/Tile), 8 NeuronCores.

Model: for each of B=8 images with a [768, 32, 32] embedding grid, compute the
mean Euclidean distance to the 3 nearest neighbors in a 20000x768 memory bank
per spatial location, then bilinear-upsample the 32x32 score map to 512x512.

Sharding: data-parallel over batch. Core b handles image b (1024 queries) with
a full bank replica. No collectives.

Per-core device program:
  - Distances via one augmented matmul chain on the TensorEngine:
      psum[q, n] = 2*q.b - q2[q] - b2[n] = -d2[q, n]
    with queries stationary ([128,128] fp32r tiles over 768 + 2 aug rows) and
    the bank streamed in 500-column chunks (fp32r => 1 cycle/row at N>=512).
  - Top-3 via the DVE max8 instruction: per-chunk top-8 of -d2 (sorted desc),
    then a final max8 over all 320 candidates per query.
  - score = mean(sqrt(max(d2,1e-12))) over top-3, via ACT Sqrt(scale=-1/9)
    with accum_out.
  - Bilinear 32->512 upsample = R @ S @ R^T, two small matmuls on-device.
"""

import numpy as np

B, E, HL, WL = 8, 768, 32, 32
N_BANK = 20000
Q = HL * WL            # 1024 queries per image
QT = Q // 128          # 8 query tiles
KC = E // 128          # 6 contraction chunks of 128
OUT = 512
NGROUPS = 10           # bank column groups
GW = N_BANK // NGROUPS # 2000 columns per group
CW = 500               # matmul moving-chunk width (one PSUM bank)
NCH = GW // CW         # 4 chunks per group
NCAND = NGROUPS * NCH * 8  # 320 candidate slots per query

_CACHE = {}


def _build_nc():
    import concourse.bass as bass
    import concourse.bacc as bacc
    import concourse.mybir as mybir
    from concourse.tile import TileContext

    f32 = mybir.dt.float32
    f32r = mybir.dt.float32r

    nc = bacc.Bacc("TRN2", target_bir_lowering=False, debug=False)

    qk_d = nc.dram_tensor("qk", [KC, 128, Q], f32r, kind="ExternalInput")
    qaug_d = nc.dram_tensor("qaug", [2, Q], f32r, kind="ExternalInput")
    bankt_d = nc.dram_tensor("bankt", [NGROUPS, KC, 128, GW], f32r, kind="ExternalInput")
    baug_d = nc.dram_tensor("baug", [2, N_BANK], f32r, kind="ExternalInput")
    rt_d = nc.dram_tensor("rt", [32, OUT], f32, kind="ExternalInput")
    out_d = nc.dram_tensor("out", [OUT, OUT], f32, kind="ExternalOutput")

    with TileContext(nc) as tc:
        with (
            tc.tile_pool(name="qpool", bufs=1) as qpool,
            tc.tile_pool(name="bpool", bufs=2) as bpool,
            tc.tile_pool(name="cpool", bufs=1) as cpool,
            tc.tile_pool(name="spool", bufs=2) as spool,
            tc.tile_pool(name="ppool", bufs=8, space=bass.MemorySpace.PSUM) as ppool,
            tc.tile_pool(name="dpool", bufs=1, space=bass.MemorySpace.DRAM) as dpool,
        ):
            # ---- persistent loads ----
            qk_sb = []
            for k in range(KC):
                t_ = qpool.tile([128, Q], f32r, tag=f"qk{k}")
                nc.sync.dma_start(t_[:], qk_d[k])
                qk_sb.append(t_)
            qaug_sb = qpool.tile([2, Q], f32r, tag="qaug")
            nc.sync.dma_start(qaug_sb[:], qaug_d[:])
            rt_sb = qpool.tile([32, OUT], f32, tag="rt")
            nc.sync.dma_start(rt_sb[:], rt_d[:])

            cand = [
                cpool.tile([128, NCAND], f32, tag=f"cand{t}", name=f"cand{t}")
                for t in range(QT)
            ]
            scores_sb = cpool.tile([128, QT], f32, tag="scores")

            # ---- distance + per-chunk top-8 ----
            for g in range(NGROUPS):
                bk = bpool.tile([128, KC, GW], f32r, tag="bank")
                for k in range(KC):
                    nc.sync.dma_start(bk[:, k, :], bankt_d[g, k])
                ba = bpool.tile([2, GW], f32r, tag="baug")
                nc.sync.dma_start(ba[:], baug_d[:, g * GW:(g + 1) * GW])
                for t in range(QT):
                    for c in range(NCH):
                        ps = ppool.tile([128, 512], f32, tag="ps")
                        for k in range(KC):
                            nc.tensor.matmul(
                                ps[:, :CW],
                                qk_sb[k][:, t * 128:(t + 1) * 128],
                                bk[:, k, c * CW:(c + 1) * CW],
                                start=(k == 0), stop=False,
                            )
                        nc.tensor.matmul(
                            ps[:, :CW],
                            qaug_sb[:, t * 128:(t + 1) * 128],
                            ba[:, c * CW:(c + 1) * CW],
                            start=False, stop=True,
                        )
                        nc.vector.max(
                            cand[t][:, (g * NCH + c) * 8:(g * NCH + c + 1) * 8],
                            ps[:, :CW],
                        )

            # ---- final top-3 -> mean distance per query ----
            for t in range(QT):
                fin8 = spool.tile([128, 8], f32, tag="fin8")
                nc.vector.max(fin8[:], cand[t][:])
                v3 = spool.tile([128, 3], f32, tag="v3")
                nc.vector.tensor_scalar_min(v3[:], fin8[:, 0:3], -1e-12)
                d3 = spool.tile([128, 3], f32, tag="d3")
                nc.scalar.activation(
                    d3[:], v3[:], mybir.ActivationFunctionType.Sqrt,
                    scale=-1.0 / 9.0, accum_out=scores_sb[:, t:t + 1],
                )

            # ---- re-layout scores [128, 8] -> S [32, 32] via DRAM bounce ----
            # query index q = t*128 + p; (h, w) = (q // 32, q % 32)
            dscr = dpool.tile([HL, WL], f32, tag="dscr")
            nc.sync.dma_start(
                dscr[:].rearrange("(t x) w -> (x w) t", t=QT), scores_sb[:]
            )
            st_sb = spool.tile([32, 32], f32, tag="st")  # st[w, h] = S[h, w]
            nc.sync.dma_start(st_sb[:], dscr[:].rearrange("h w -> w h"))

            # ---- bilinear upsample: out = R @ S @ R^T ----
            psu = ppool.tile([128, 512], f32, tag="ps")
            nc.tensor.matmul(psu[:32, :], st_sb[:], rt_sb[:],
                             start=True, stop=True)
            u_sb = spool.tile([32, OUT], f32, tag="u")
            nc.vector.tensor_copy(u_sb[:], psu[:32, :])
            for i in range(4):
                po = ppool.tile([128, 512], f32, tag="ps")
                nc.tensor.matmul(po[:], rt_sb[:, i * 128:(i + 1) * 128],
                                 u_sb[:], start=True, stop=True)
                osb = spool.tile([128, OUT], f32, tag="osb")
                nc.vector.tensor_copy(osb[:], po[:])
                nc.sync.dma_start(out_d[i * 128:(i + 1) * 128, :], osb[:])

    nc.compile()
    return nc


def _resize_matrix(n_in: int, n_out: int) -> np.ndarray:
    """Bilinear (half-pixel, edge-clamped) interpolation matrix [n_out, n_in].
    Matches jax.image.resize(method='bilinear') for upsampling."""
    R = np.zeros((n_out, n_in), dtype=np.float64)
    scale = n_in / n_out
    for i in range(n_out):
        src = (i + 0.5) * scale - 0.5
        a0 = int(np.floor(src))
        w = src - a0
        a0c = min(max(a0, 0), n_in - 1)
        a1c = min(max(a0 + 1, 0), n_in - 1)
        R[i, a0c] += 1.0 - w
        R[i, a1c] += w
    return R.astype(np.float32)


def _prep_inputs(embeddings: np.ndarray, bank: np.ndarray):
    """Host-side layout prep. Returns per-core input maps."""
    f = np.float32
    emb = np.asarray(embeddings, dtype=f)
    bank = np.asarray(bank, dtype=f)

    # queries: [B, E, HL, WL] -> qT [B, E, Q] (E-major for the stationary side)
    qT = emb.reshape(B, E, Q)
    q2 = np.einsum("beq,beq->bq", qT, qT)              # [B, Q]
    qk_all = np.ascontiguousarray(qT.reshape(B, KC, 128, Q))
    qaug_all = np.stack(
        [q2, np.ones((B, Q), dtype=f)], axis=1
    ).astype(f)                                         # [B, 2, Q]

    bankT2 = np.ascontiguousarray((2.0 * bank).T)       # [E, N]
    bankt = np.ascontiguousarray(
        bankT2.reshape(KC, 128, NGROUPS, GW).transpose(2, 0, 1, 3)
    )                                                   # [NGROUPS, KC, 128, GW]
    b2 = np.einsum("ne,ne->n", bank, bank)
    baug = np.stack([-np.ones(N_BANK, dtype=f), -b2]).astype(f)  # [2, N]

    rt = np.ascontiguousarray(_resize_matrix(HL, OUT).T)  # [32, 512]

    in_maps = [
        {
            "qk": qk_all[b],
            "qaug": np.ascontiguousarray(qaug_all[b]),
            "bankt": bankt,
            "baug": baug,
            "rt": rt,
        }
        for b in range(B)
    ]
    return in_maps


def kernel(embeddings, bank, out_size, _trace=False, _trace_kwargs=None):
    from concourse import bass_utils

    assert int(out_size) == OUT
    if "nc" not in _CACHE:
        _CACHE["nc"] = _build_nc()
    nc = _CACHE["nc"]

    in_maps = _prep_inputs(np.asarray(embeddings), np.asarray(bank))
    res = bass_utils.run_bass_kernel_spmd(
        nc, in_maps, core_ids=list(range(B)), trace=_trace,
        **(_trace_kwargs or {}),
    )
    _CACHE["last_results"] = res
    out = np.stack([res.results[b]["out"] for b in range(B)])
    return out.reshape(B, 1, OUT, OUT).astype(np.float32)

